# revision 5
# baseline (speedup 1.0000x reference)
"""Trainium2 Bass kernel for nn_MultiHeadAttention_41884521070801.

Sharding: tensor-parallel over heads (4 heads/core) x data-parallel over
batch (B=2) => 8 cores. Each core computes, for its batch element and its
4 heads: QKV projections (+RoPE), causal softmax attention (flash-style,
transposed-scores layout so no transposes are needed on-device), and its
partial output projection (rows of Wo^T). Host sums the 4 partial outputs
per batch element.

All matmuls run in bf16 with fp32 PSUM accumulation. RoPE and softmax
statistics are computed in fp32.
"""

import math

import numpy as np
import ml_dtypes

import concourse.bacc as bacc
import concourse.tile as tile
from concourse import mybir
from concourse.bass_utils import run_bass_kernel_spmd

N_CORES = 8
B = 2
S = 2048
D = 2048
H = 16
HD = 128          # head dim
HLOC = 4          # heads per core
DLOC = HLOC * HD  # 512, per-core slice of the concat-head dim
QCH = 512         # q chunk size
NQC = S // QCH    # 4
NKB = S // 128    # 16 k-blocks
NEB = D // 128    # 16 e-blocks (contraction blocks for projections)
ROPE_THETA = 10000.0
NEG = -1.0e30

F32 = mybir.dt.float32
BF16 = mybir.dt.bfloat16

_BUILD_CACHE = {}


def _emit_consts(nc, tc, pools, tensors):
    """Emit the one-time constant/weight loads."""
    (consts, resid, xc_pool, ps_pool, work, p_pool, rb_pool, oc_pool,
     qcur_pool, ocur_pool) = pools
    (xT, wqT, wkT, wvT, woT, cosT, sinT, rT, amB, ctri, outp) = tensors
    if True:
        consts.wq = consts.tile([128, NEB, DLOC], BF16, tag="wq", name="wq")
        consts.wk = consts.tile([128, NEB, DLOC], BF16, tag="wk", name="wk")
        consts.wv = consts.tile([128, NEB, DLOC], BF16, tag="wv", name="wv")
        consts.wo = consts.tile([128, HLOC, D], BF16, tag="wo", name="wo")
        nc.sync.dma_start(out=consts.wq, in_=wqT[:].rearrange("(e p) d -> p e d", p=128))
        nc.sync.dma_start(out=consts.wk, in_=wkT[:].rearrange("(e p) d -> p e d", p=128))
        nc.sync.dma_start(out=consts.wv, in_=wvT[:].rearrange("(e p) d -> p e d", p=128))
        nc.sync.dma_start(out=consts.wo, in_=woT[:].rearrange("(h p) d -> p h d", p=128))
        consts.cos = consts.tile([128, S], F32, tag="cos", name="cos")
        consts.sin = consts.tile([128, S], F32, tag="sin", name="sin")
        nc.sync.dma_start(out=consts.cos, in_=cosT[:])
        nc.sync.dma_start(out=consts.sin, in_=sinT[:])
        consts.rT = consts.tile([128, HD], F32, tag="rT", name="rTs")
        nc.sync.dma_start(out=consts.rT, in_=rT[:])
        consts.amB = consts.tile([128, NKB], F32, tag="amB", name="amBs")
        nc.sync.dma_start(out=consts.amB, in_=amB[:])
        consts.ctri = consts.tile([128, 4, QCH], F32, tag="ctri", name="ctri")
        nc.sync.dma_start(out=consts.ctri, in_=ctri[:].rearrange("p (j q) -> p j q", j=4))
        consts.ones_bf = consts.tile([128, 1], BF16, tag="ones_bf", name="ones_bf")
        nc.vector.memset(consts.ones_bf, 1.0)
        consts.ones_row = consts.tile([1, 128], F32, tag="ones_row", name="ones_row")
        nc.vector.memset(consts.ones_row, 1.0)
        # persistent activations (K and V must stay for the whole pass)
        consts.kro = [resid.tile([128, S], BF16, tag=f"kro{h}", name=f"kro{h}")
                      for h in range(HLOC)]
        consts.v = [resid.tile([128, DLOC], BF16, tag=f"v{kb}", name=f"v{kb}")
                    for kb in range(NKB)]


def _emit_body(nc, tc, pools, tensors):
    """Emit one full forward pass (consts already emitted)."""
    (consts, resid, xc_pool, ps_pool, work, p_pool, rb_pool, oc_pool,
     qcur_pool, ocur_pool) = pools
    (xT, wqT, wkT, wvT, woT, cosT, sinT, rT, amB, ctri, outp) = tensors

    wq, wk, wv, wo = consts.wq, consts.wk, consts.wv, consts.wo
    cos_s, sin_s, rT_s, amB_s, ctri_s = (
        consts.cos, consts.sin, consts.rT, consts.amB, consts.ctri)
    kro, v_s = consts.kro, consts.v

    def rope(src_ps, dst_ap, qc):
        """dst_ap (bf16 [128, QCH]) = rope of src_ps ([128 hd, QCH] psum f32)."""
        qf = work.tile([128, QCH], F32, tag="ropef", name="ropef")
        nc.scalar.copy(qf, src_ps)
        rot = ps_pool.tile([128, QCH], F32, tag="ps", name="ps")
        nc.tensor.matmul(rot, lhsT=rT_s, rhs=qf, start=True, stop=True)
        t1 = work.tile([128, QCH], F32, tag="ropet1", name="ropet1")
        nc.vector.tensor_mul(t1, qf, cos_s[:, qc * QCH:(qc + 1) * QCH])
        t2 = work.tile([128, QCH], F32, tag="ropet2", name="ropet2")
        nc.vector.tensor_mul(t2, rot, sin_s[:, qc * QCH:(qc + 1) * QCH])
        nc.vector.tensor_add(dst_ap, t1, t2)

    for qc in range(NQC):
        # ---- load x^T chunk: 16 tiles [128 e, 512 q] ----
        xc = []
        for e in range(NEB):
            t = xc_pool.tile([128, QCH], BF16, tag="xc", name="xc")
            nc.sync.dma_start(
                out=t, in_=xT[e * 128:(e + 1) * 128, qc * QCH:(qc + 1) * QCH])
            xc.append(t)

        # ---- Q^T and K^T projections (one head = one 128-row block) ----
        qcur = []
        for h in range(HLOC):
            qt = qcur_pool.tile([128, QCH], BF16, tag="qcur", name="qcur")
            qcur.append(qt)
            pp = ps_pool.tile([128, QCH], F32, tag="ps", name="ps")
            for e in range(NEB):
                nc.tensor.matmul(
                    pp, lhsT=wq[:, e, h * HD:(h + 1) * HD], rhs=xc[e],
                    start=(e == 0), stop=(e == NEB - 1))
            rope(pp, qt[:], qc)
        for h in range(HLOC):
            pp = ps_pool.tile([128, QCH], F32, tag="ps", name="ps")
            for e in range(NEB):
                nc.tensor.matmul(
                    pp, lhsT=wk[:, e, h * HD:(h + 1) * HD], rhs=xc[e],
                    start=(e == 0), stop=(e == NEB - 1))
            rope(pp, kro[h][:, qc * QCH:(qc + 1) * QCH], qc)

        # ---- V (natural [k, d] layout) ----
        for kb4 in range(4):
            kb = qc * 4 + kb4
            pp = ps_pool.tile([128, DLOC], F32, tag="ps", name="ps")
            for e in range(NEB):
                nc.tensor.matmul(
                    pp, lhsT=xc[e][:, kb4 * 128:(kb4 + 1) * 128], rhs=wv[:, e, :],
                    start=(e == 0), stop=(e == NEB - 1))
            nc.scalar.copy(v_s[kb], pp)

        # ---- attention for this q chunk (flash-style over k-blocks) ----
        nkb = 4 * qc + 4
        ocur = []
        for h in range(HLOC):
            ops = ps_pool.tile([128, QCH], F32, tag="ps", name="ps")
            sps = ps_pool.tile([1, QCH], F32, tag="ps", name="ps")
            for kb in range(nkb):
                s_ps = ps_pool.tile([128, QCH], F32, tag="ps", name="ps")
                nc.tensor.matmul(
                    s_ps, lhsT=kro[h][:, kb * 128:(kb + 1) * 128], rhs=qcur[h],
                    start=True, stop=True)
                if kb >= 4 * qc:
                    nc.vector.tensor_add(s_ps, s_ps, ctri_s[:, kb - 4 * qc, :])
                p_sb = p_pool.tile([128, QCH], BF16, tag="p", name="p")
                nc.scalar.activation(
                    p_sb, s_ps, mybir.ActivationFunctionType.Exp,
                    bias=amB_s[:, kb:kb + 1], scale=1.0)
                nc.tensor.matmul(
                    ops, lhsT=v_s[kb][:, h * HD:(h + 1) * HD], rhs=p_sb,
                    start=(kb == 0), stop=(kb == nkb - 1), skip_group_check=True)
                nc.tensor.matmul(
                    sps, lhsT=consts.ones_bf, rhs=p_sb,
                    start=(kb == 0), stop=(kb == nkb - 1), skip_group_check=True)
            # normalize: o = ops * (1/sums), broadcast along partitions via
            # a K=1 outer-product matmul
            r_row = rb_pool.tile([1, QCH], F32, tag="rrow", name="rrow")
            nc.vector.reciprocal(r_row, sps)
            rb_ps = ps_pool.tile([128, QCH], F32, tag="ps", name="ps")
            nc.tensor.matmul(rb_ps, lhsT=consts.ones_row, rhs=r_row,
                             start=True, stop=True)
            rb_sb = rb_pool.tile([128, QCH], F32, tag="rb", name="rb")
            nc.vector.tensor_copy(rb_sb, rb_ps)
            ot = ocur_pool.tile([128, QCH], BF16, tag="ocur", name="ocur")
            ocur.append(ot)
            nc.vector.tensor_mul(ot[:], ops, rb_sb)

        # ---- output projection for this chunk ----
        for qb4 in range(QCH // 128):
            qb = qc * 4 + qb4
            for ec in range(D // QCH):
                op_ps = ps_pool.tile([128, QCH], F32, tag="ps", name="ps")
                for h in range(HLOC):
                    nc.tensor.matmul(
                        op_ps,
                        lhsT=ocur[h][:, qb4 * 128:(qb4 + 1) * 128],
                        rhs=wo[:, h, ec * QCH:(ec + 1) * QCH],
                        start=(h == 0), stop=(h == HLOC - 1))
                oc = oc_pool.tile([128, QCH], F32, tag="oc", name="oc")
                nc.vector.tensor_copy(oc, op_ps)
                nc.sync.dma_start(
                    out=outp[qb * 128:(qb + 1) * 128, ec * QCH:(ec + 1) * QCH],
                    in_=oc)


def build_nc(repeat=1):
    key = repeat
    if key in _BUILD_CACHE:
        return _BUILD_CACHE[key]
    nc = bacc.Bacc("TRN2", target_bir_lowering=False, debug=False,
                   num_devices=N_CORES)
    xT = nc.dram_tensor("xT", [D, S], BF16, kind="ExternalInput")
    wqT = nc.dram_tensor("wqT", [D, DLOC], BF16, kind="ExternalInput")
    wkT = nc.dram_tensor("wkT", [D, DLOC], BF16, kind="ExternalInput")
    wvT = nc.dram_tensor("wvT", [D, DLOC], BF16, kind="ExternalInput")
    woT = nc.dram_tensor("woT", [DLOC, D], BF16, kind="ExternalInput")
    cosT = nc.dram_tensor("cosT", [HD, S], F32, kind="ExternalInput")
    sinT = nc.dram_tensor("sinT", [HD, S], F32, kind="ExternalInput")
    rT = nc.dram_tensor("rT", [HD, HD], F32, kind="ExternalInput")
    amB = nc.dram_tensor("amB", [128, NKB], F32, kind="ExternalInput")
    ctri = nc.dram_tensor("ctri", [128, 4 * QCH], F32, kind="ExternalInput")
    outp = nc.dram_tensor("outp", [S, D], F32, kind="ExternalOutput")
    tensors = (xT, wqT, wkT, wvT, woT, cosT, sinT, rT, amB, ctri, outp)

    from contextlib import ExitStack
    with tile.TileContext(nc) as tc, ExitStack() as ctx:
        consts = ctx.enter_context(tc.tile_pool(name="consts", bufs=1))
        resid = ctx.enter_context(tc.tile_pool(name="resid", bufs=1))
        xc_pool = ctx.enter_context(tc.tile_pool(name="xc", bufs=16))
        ps_pool = ctx.enter_context(tc.tile_pool(name="ps", bufs=8, space="PSUM"))
        work = ctx.enter_context(tc.tile_pool(name="work", bufs=2))
        p_pool = ctx.enter_context(tc.tile_pool(name="p", bufs=6))
        rb_pool = ctx.enter_context(tc.tile_pool(name="rb", bufs=2))
        oc_pool = ctx.enter_context(tc.tile_pool(name="oc", bufs=3))
        qcur_pool = ctx.enter_context(tc.tile_pool(name="qcur", bufs=8))
        ocur_pool = ctx.enter_context(tc.tile_pool(name="ocur", bufs=8))
        pools = (consts, resid, xc_pool, ps_pool, work, p_pool, rb_pool,
                 oc_pool, qcur_pool, ocur_pool)
        _emit_consts(nc, tc, pools, tensors)
        if repeat == 1:
            _emit_body(nc, tc, pools, tensors)
        else:
            with tc.For_i(0, repeat, 1):
                _emit_body(nc, tc, pools, tensors)
    nc.compile()
    _BUILD_CACHE[key] = nc
    return nc


def make_core_inputs(hidden_states, attention_mask, Wq, Wk, Wv, Wo):
    """Host-side prep: returns list of 8 in_maps."""
    f32 = np.float32
    bf16 = ml_dtypes.bfloat16
    hidden_states = np.asarray(hidden_states, dtype=f32)
    attention_mask = np.asarray(attention_mask, dtype=f32)
    Wq = np.asarray(Wq, dtype=f32)
    Wk = np.asarray(Wk, dtype=f32)
    Wv = np.asarray(Wv, dtype=f32)
    Wo = np.asarray(Wo, dtype=f32)

    # rope tables, [hd, S] layout
    invf = 1.0 / (ROPE_THETA ** (np.arange(0, HD, 2, dtype=f32) / HD))
    t = np.arange(S, dtype=f32)
    fr = t[:, None] * invf[None, :]            # [S, hd/2]
    emb = np.concatenate([fr, fr], axis=-1)    # [S, hd]
    cosT = np.cos(emb).T.astype(f32).copy()    # [hd, S]
    sinT = np.sin(emb).T.astype(f32).copy()

    # rotate-half matrix: (R @ x)[i] = -x[i+64] (i<64), x[i-64] (i>=64)
    R = np.zeros((HD, HD), dtype=f32)
    half = HD // 2
    for i in range(half):
        R[i, i + half] = -1.0
        R[i + half, i] = 1.0
    rT = R.T.copy()

    # causal additive mask variants for diagonal k-blocks, [128, 4*512]
    ctri = np.zeros((128, 4, QCH), dtype=f32)
    p = np.arange(128)[:, None]
    c = np.arange(QCH)[None, :]
    for j in range(4):
        qrel = c - 128 * j
        ctri[:, j, :] = np.where((qrel < 0) | (p > qrel), NEG, 0.0)
    ctri = ctri.reshape(128, 4 * QCH)

    scale = 1.0 / math.sqrt(HD)
    in_maps = []
    for core in range(N_CORES):
        b = core // (N_CORES // B)
        hg = core % (N_CORES // B)
        rows = slice(hg * DLOC, (hg + 1) * DLOC)
        amv = np.where(attention_mask[b] == 0, NEG, attention_mask[b]).astype(f32)
        in_maps.append({
            "xT": hidden_states[b].T.astype(bf16),
            "wqT": (Wq[rows, :] * scale).T.astype(bf16),
            "wkT": Wk[rows, :].T.astype(bf16),
            "wvT": Wv[rows, :].T.astype(bf16),
            "woT": Wo[:, rows].T.astype(bf16),
            "cosT": cosT,
            "sinT": sinT,
            "rT": rT,
            "amB": amv.reshape(NKB, 128).T.copy(),
            "ctri": ctri,
        })
    return in_maps


def kernel(**inputs):
    nc = build_nc()
    in_maps = make_core_inputs(**inputs)
    res = run_bass_kernel_spmd(nc, in_maps, list(range(N_CORES)))
    out = np.zeros((B, S, D), dtype=np.float32)
    ncb = N_CORES // B
    for core in range(N_CORES):
        out[core // ncb] += res.results[core]["outp"]
    return out
